# revision 1
# baseline (speedup 1.0000x reference)
"""Trainium2 Bass kernel for nn_ChatBlock (attention + top-2-of-8 MoE block).

Strategy
--------
Data-parallel over batch: 64 batches -> 8 NeuronCores x 8 batches. No
collectives; each core runs the full block on its shard.

Per batch (T=384 tokens, C=256):
  - activations live token-major ([128 tok, C] tiles) for rmsnorm /
    softmax / routing (free-axis reductions), and are PE-transposed to
    feature-major ([C, 384]) to serve as matmul moving operands.
  - all heavy matmuls run in bf16 with f32 PSUM accumulation.
  - attention: scores computed token-major (q in partitions), masked
    softmax without max-subtraction (scores are provably tiny), probs
    normalized token-major then PE-transposed for the att @ v matmul.
  - router runs in fp32-equivalent precision via a bf16 hi/lo split of
    h2 so top-2 expert selection matches the fp32 reference.
  - MoE v1: dense-masked — every expert computes every token; the
    per-token combine weight (0 for unselected experts) is applied in
    the token-major accumulation.
"""

import sys
from contextlib import ExitStack

sys.path.insert(0, "/opt/trn_rl_repo")

import ml_dtypes
import numpy as np

import concourse.bass as bass
import concourse.mybir as mybir
import concourse.tile as tile
from concourse import bacc
from concourse.bass_utils import run_bass_kernel_spmd

F32 = mybir.dt.float32
BF16 = mybir.dt.bfloat16
AF = mybir.ActivationFunctionType
ALU = mybir.AluOpType

B, T, C = 64, 384, 256
NH, NKV = 4, 2
HD = C // NH            # 64
NE, TOPK = 8, 2
HID = int(8 / 3 * C)    # 682
HIDP = 768              # padded to 6*128
EPS_RMS = 1e-6
N_CORES = 8
B_LOC = B // N_CORES    # 8 batches per core
NT = T // 128           # 3 token chunks per batch
KT = C // 128           # 2 feature chunks
MT = HIDP // 128        # 6 hidden chunks
NEG = -1.0e30


def _mktile(pool, shape, dtype, tag, **kw):
    return pool.tile(shape, dtype, tag=tag, name=tag, **kw)


def _build(n_batches=B_LOC):
    nc = bacc.Bacc("TRN2", target_bir_lowering=False)

    x_d = nc.dram_tensor("x", [n_batches, T, C], F32, kind="ExternalInput")
    wqt_d = nc.dram_tensor("wqt", [C, C], BF16, kind="ExternalInput")
    wkt_d = nc.dram_tensor("wkt", [C, NKV * HD], BF16, kind="ExternalInput")
    wvt_d = nc.dram_tensor("wvt", [C, NKV * HD], BF16, kind="ExternalInput")
    wot_d = nc.dram_tensor("wot", [C, C], BF16, kind="ExternalInput")
    rt_hi_d = nc.dram_tensor("rt_hi", [C, NE], BF16, kind="ExternalInput")
    rt_lo_d = nc.dram_tensor("rt_lo", [C, NE], BF16, kind="ExternalInput")
    w1t_d = nc.dram_tensor("w1t", [NE, C, HIDP], BF16, kind="ExternalInput")
    w3t_d = nc.dram_tensor("w3t", [NE, C, HIDP], BF16, kind="ExternalInput")
    w2t_d = nc.dram_tensor("w2t", [NE, HIDP, C], BF16, kind="ExternalInput")
    cosq_d = nc.dram_tensor("cosq", [128, T], F32, kind="ExternalInput")
    sinq_d = nc.dram_tensor("sinq", [128, T], F32, kind="ExternalInput")
    cosk_d = nc.dram_tensor("cosk", [128, T], F32, kind="ExternalInput")
    sink_d = nc.dram_tensor("sink", [128, T], F32, kind="ExternalInput")
    prot_d = nc.dram_tensor("prot", [128, 128], BF16, kind="ExternalInput")
    ident_d = nc.dram_tensor("ident", [128, 128], BF16, kind="ExternalInput")
    mdiag_d = nc.dram_tensor("mdiag", [128, 128], F32, kind="ExternalInput")
    out_d = nc.dram_tensor("out", [n_batches, T, C], F32, kind="ExternalOutput")

    with tile.TileContext(nc) as tc, ExitStack() as ctx:
        cpool = ctx.enter_context(tc.tile_pool(name="const", bufs=1))

        def cload(dram, shape, dtype, tag, ap=None):
            t = cpool.tile(shape, dtype, tag=tag, name=tag)
            nc.sync.dma_start(t[:], dram[:] if ap is None else ap)
            return t

        # persistent weights / tables (partition-major layouts)
        wq_sb = cload(wqt_d, [128, KT, C], BF16, "wq",
                      wqt_d.rearrange("(t p) m -> p t m", p=128))
        wk_sb = cload(wkt_d, [128, KT, NKV * HD], BF16, "wk",
                      wkt_d.rearrange("(t p) m -> p t m", p=128))
        wv_sb = cload(wvt_d, [128, KT, NKV * HD], BF16, "wv",
                      wvt_d.rearrange("(t p) m -> p t m", p=128))
        wo_sb = cload(wot_d, [128, KT, C], BF16, "wo",
                      wot_d.rearrange("(t p) m -> p t m", p=128))
        rhi_sb = cload(rt_hi_d, [128, KT, NE], BF16, "rhi",
                       rt_hi_d.rearrange("(t p) m -> p t m", p=128))
        rlo_sb = cload(rt_lo_d, [128, KT, NE], BF16, "rlo",
                       rt_lo_d.rearrange("(t p) m -> p t m", p=128))
        w1_sb = cload(w1t_d, [128, NE * KT, HIDP], BF16, "w1",
                      w1t_d.rearrange("e (t p) h -> p (e t) h", p=128))
        w3_sb = cload(w3t_d, [128, NE * KT, HIDP], BF16, "w3",
                      w3t_d.rearrange("e (t p) h -> p (e t) h", p=128))
        w2_sb = cload(w2t_d, [128, NE * MT, C], BF16, "w2",
                      w2t_d.rearrange("e (t p) m -> p (e t) m", p=128))
        cosq_sb = cload(cosq_d, [128, T], F32, "cosq")
        sinq_sb = cload(sinq_d, [128, T], F32, "sinq")
        cosk_sb = cload(cosk_d, [128, T], F32, "cosk")
        sink_sb = cload(sink_d, [128, T], F32, "sink")
        prot_sb = cload(prot_d, [128, 128], BF16, "prot")
        ident_sb = cload(ident_d, [128, 128], BF16, "ident")
        mdiag_sb = cload(mdiag_d, [128, 128], F32, "mdiag")
        eps_sb = _mktile(cpool, [128, 1], F32, "eps")
        nc.gpsimd.memset(eps_sb[:], EPS_RMS)

        # working pools
        sb = ctx.enter_context(tc.tile_pool(name="sb", bufs=2))
        sbs = ctx.enter_context(tc.tile_pool(name="sbs", bufs=4))   # small stats
        sbe = ctx.enter_context(tc.tile_pool(name="sbe", bufs=4))   # attn probs
        sbu = ctx.enter_context(tc.tile_pool(name="sbu", bufs=7))   # moe hidden
        psA = ctx.enter_context(tc.tile_pool(name="psA", bufs=2, space="PSUM"))
        psY = ctx.enter_context(tc.tile_pool(name="psY", bufs=2, space="PSUM"))
        psT = ctx.enter_context(tc.tile_pool(name="psT", bufs=2, space="PSUM"))
        psO = ctx.enter_context(tc.tile_pool(name="psO", bufs=2, space="PSUM"))

        def rmsnorm_bf16(x_t, h_bf, tag):
            """x_t [128, NT, C] f32 -> h_bf [128, NT, C] bf16 (per-token scale).

            ln weight is pre-folded into the following matmul weights.
            """
            for n in range(NT):
                sq = _mktile(sbs, [128, C], F32, "sq")
                ss = _mktile(sbs, [128, 1], F32, tag + "ss")
                nc.scalar.activation(sq[:], x_t[:, n, :], AF.Square, accum_out=ss[:])
                std = _mktile(sbs, [128, 1], F32, tag + "std")
                nc.scalar.activation(std[:], ss[:], AF.Sqrt, scale=1.0 / C,
                                     bias=eps_sb[:, :1])
                inv = _mktile(sbs, [128, 1], F32, tag + "inv")
                nc.vector.reciprocal(inv[:], std[:])
                nc.vector.tensor_scalar_mul(h_bf[:, n, :], x_t[:, n, :], inv[:, :1])

        def transpose_128(src_ap, dst_ap, dtype=BF16):
            """PE-transpose one [128, 128] block via identity."""
            pt = _mktile(psT, [128, 128], dtype, "tr")
            nc.tensor.transpose(pt[:], src_ap, ident_sb[:])
            nc.vector.tensor_copy(dst_ap, pt[:])

        for b in range(n_batches):
            # ---- load x (token-major [128, NT, C]) ----
            x_sb = _mktile(sb, [128, NT, C], F32, "x")
            nc.sync.dma_start(x_sb[:], x_d[b].rearrange("(n p) c -> p n c", p=128))

            # ---- rmsnorm 1 ----
            h_bf = _mktile(sb, [128, NT, C], BF16, "h")
            rmsnorm_bf16(x_sb, h_bf, "r1")

            # ---- transpose h -> feature-major hT [C, T] (2 tiles) ----
            hT = [_mktile(sb, [128, T], BF16, f"hT{kt}") for kt in range(KT)]
            for kt in range(KT):
                for n in range(NT):
                    transpose_128(h_bf[:, n, bass.ts(kt, 128)],
                                  hT[kt][:, bass.ts(n, 128)])

            # ---- QKV ----
            # qT feature-major [C, T]: 2 psum tiles
            qr = []
            for mt in range(KT):
                pq = _mktile(psA, [128, T], F32, "A")
                for kt in range(KT):
                    nc.tensor.matmul(pq[:], wq_sb[:, kt, bass.ts(mt, 128)],
                                     hT[kt][:], start=(kt == 0), stop=(kt == KT - 1))
                q_sb = _mktile(sb, [128, T], BF16, f"q{mt}")
                nc.scalar.copy(q_sb[:], pq[:])
                qr.append(q_sb)
            # kT feature-major [128, T] (2 kv heads x 64)
            pk = _mktile(psA, [128, T], F32, "A")
            for kt in range(KT):
                nc.tensor.matmul(pk[:], wk_sb[:, kt, :], hT[kt][:],
                                 start=(kt == 0), stop=(kt == KT - 1))
            k_sb = _mktile(sb, [128, T], BF16, "k")
            nc.scalar.copy(k_sb[:], pk[:])
            # v token-major [128, NT, NKV*HD]
            v_sb = _mktile(sb, [128, NT, NKV * HD], BF16, "v")
            for n in range(NT):
                pv = _mktile(psT, [128, NKV * HD], F32, "tr")
                for kt in range(KT):
                    nc.tensor.matmul(pv[:], hT[kt][:, bass.ts(n, 128)],
                                     wv_sb[:, kt, :], start=(kt == 0),
                                     stop=(kt == KT - 1))
                nc.scalar.copy(v_sb[:, n, :], pv[:])

            # ---- RoPE (feature-major): q' = q*cos + rot(q)*sin ----
            # cos/sin for q pre-scaled by 1/sqrt(HD).
            q2 = []
            for mt in range(KT):
                pr = _mktile(psA, [128, T], F32, "A")
                nc.tensor.matmul(pr[:], prot_sb[:], qr[mt][:])
                t1 = _mktile(sbs, [128, T], F32, "ropet1")
                nc.vector.tensor_tensor(t1[:], qr[mt][:], cosq_sb[:], op=ALU.mult)
                t2 = _mktile(sbs, [128, T], F32, "ropet2")
                nc.vector.tensor_tensor(t2[:], pr[:], sinq_sb[:], op=ALU.mult)
                q_sb2 = _mktile(sb, [128, T], BF16, f"q2{mt}")
                nc.vector.tensor_tensor(q_sb2[:], t1[:], t2[:], op=ALU.add)
                q2.append(q_sb2)
            # k rope + duplicate each kv head to both partition halves so
            # lhsT/rhs partition bases match in the scores matmul.
            pr = _mktile(psA, [128, T], F32, "A")
            nc.tensor.matmul(pr[:], prot_sb[:], k_sb[:])
            t1 = _mktile(sbs, [128, T], F32, "ropet1")
            nc.vector.tensor_tensor(t1[:], k_sb[:], cosk_sb[:], op=ALU.mult)
            t2 = _mktile(sbs, [128, T], F32, "ropet2")
            nc.vector.tensor_tensor(t2[:], pr[:], sink_sb[:], op=ALU.mult)
            k2 = _mktile(sb, [128, T], BF16, "k2")
            nc.vector.tensor_tensor(k2[:], t1[:], t2[:], op=ALU.add)
            kdup = []
            for g in range(NKV):
                kd = _mktile(sb, [128, T], BF16, f"kd{g}")
                nc.vector.tensor_copy(kd[:64, :], k2[bass.ts(g, 64), :])
                nc.vector.tensor_copy(kd[64:, :], k2[bass.ts(g, 64), :])
                kdup.append(kd)

            # ---- attention ----
            yT_ps = [_mktile(psY, [128, T], F32, "Y") for _ in range(KT)]
            for h in range(NH):
                g = h // 2
                hp = 64 * (h % 2)
                # probs transposed, per key-chunk: eT[kc][:, q] (bf16)
                eT = [_mktile(sbe, [128, T], BF16, f"eT{kc}") for kc in range(NT)]
                for qc in range(NT):
                    w = (qc + 1) * 128
                    ps = _mktile(psA, [128, T], F32, "A")
                    nc.tensor.matmul(
                        ps[:, :w],
                        q2[g][hp:hp + 64, bass.ts(qc, 128)],
                        kdup[g][hp:hp + 64, :w])
                    # causal mask on the diagonal 128-block only
                    nc.vector.tensor_tensor(
                        ps[:, bass.ts(qc, 128)], ps[:, bass.ts(qc, 128)],
                        mdiag_sb[:], op=ALU.add)
                    e_sb = _mktile(sbs, [128, T], F32, "e")
                    den = _mktile(sbs, [128, 1], F32, "den")
                    nc.scalar.activation(e_sb[:, :w], ps[:, :w], AF.Exp,
                                         accum_out=den[:])
                    r = _mktile(sbs, [128, 1], F32, "r")
                    nc.vector.reciprocal(r[:], den[:])
                    en = _mktile(sbs, [128, T], BF16, "en")
                    nc.vector.tensor_scalar_mul(en[:, :w], e_sb[:, :w], r[:, :1])
                    for kc in range(qc + 1):
                        transpose_128(en[:, bass.ts(kc, 128)],
                                      eT[kc][:, bass.ts(qc, 128)])
                # att @ v -> yT feature-major [64, T], heads packed in pairs
                for kc in range(NT):
                    w0 = kc * 128
                    nc.tensor.matmul(
                        yT_ps[h // 2][hp:hp + 64, w0:T],
                        v_sb[:, kc, bass.ts(g, 64)],
                        eT[kc][:, w0:T],
                        start=(kc == 0), stop=(kc == NT - 1))
            yT = []
            for mt in range(KT):
                y_sb = _mktile(sb, [128, T], BF16, f"yT{mt}")
                nc.scalar.copy(y_sb[:], yT_ps[mt][:])
                yT.append(y_sb)

            # ---- wo + residual -> x2 (token-major f32) ----
            x2_sb = _mktile(sb, [128, NT, C], F32, "x2")
            for n in range(NT):
                po = _mktile(psO, [128, C], F32, "O")
                for mt in range(KT):
                    nc.tensor.matmul(po[:], yT[mt][:, bass.ts(n, 128)],
                                     wo_sb[:, mt, :], start=(mt == 0),
                                     stop=(mt == KT - 1))
                nc.vector.tensor_tensor(x2_sb[:, n, :], po[:], x_sb[:, n, :],
                                        op=ALU.add)

            # ---- rmsnorm 2 (hi/lo split for exact-enough routing) ----
            h2 = _mktile(sb, [128, NT, C], F32, "h2")
            for n in range(NT):
                sq = _mktile(sbs, [128, C], F32, "sq")
                ss = _mktile(sbs, [128, 1], F32, "r2ss")
                nc.scalar.activation(sq[:], x2_sb[:, n, :], AF.Square,
                                     accum_out=ss[:])
                std = _mktile(sbs, [128, 1], F32, "r2std")
                nc.scalar.activation(std[:], ss[:], AF.Sqrt, scale=1.0 / C,
                                     bias=eps_sb[:, :1])
                inv = _mktile(sbs, [128, 1], F32, "r2inv")
                nc.vector.reciprocal(inv[:], std[:])
                nc.vector.tensor_scalar_mul(h2[:, n, :], x2_sb[:, n, :], inv[:, :1])
            h2hi = _mktile(sb, [128, NT, C], BF16, "h2hi")
            h2lo = _mktile(sb, [128, NT, C], BF16, "h2lo")
            for n in range(NT):
                nc.vector.tensor_copy(h2hi[:, n, :], h2[:, n, :])
                nc.vector.tensor_tensor(h2lo[:, n, :], h2[:, n, :], h2hi[:, n, :],
                                        op=ALU.subtract)

            h2T = [_mktile(sb, [128, T], BF16, f"h2T{kt}") for kt in range(KT)]
            h2Tlo = [_mktile(sb, [128, T], BF16, f"h2Tlo{kt}") for kt in range(KT)]
            for kt in range(KT):
                for n in range(NT):
                    transpose_128(h2hi[:, n, bass.ts(kt, 128)],
                                  h2T[kt][:, bass.ts(n, 128)])
                    transpose_128(h2lo[:, n, bass.ts(kt, 128)],
                                  h2Tlo[kt][:, bass.ts(n, 128)])

            # ---- router: probs + top-2 combine weights (token-major) ----
            W_sb = _mktile(sb, [128, NT, NE], F32, "W")
            for n in range(NT):
                plog = _mktile(psO, [128, NE], F32, "O")
                terms = [(h2T[kt], r, kt) for kt in range(KT)
                         for r in (rhi_sb, rlo_sb)]
                terms += [(h2Tlo[kt], rhi_sb, kt) for kt in range(KT)]
                for i, (lhs, rhs, kt) in enumerate(terms):
                    nc.tensor.matmul(plog[:], lhs[:, bass.ts(n, 128)],
                                     rhs[:, kt, :], start=(i == 0),
                                     stop=(i == len(terms) - 1))
                p_sb = _mktile(sbs, [128, NE], F32, "p")
                nc.scalar.activation(p_sb[:], plog[:], AF.Exp)
                m1 = _mktile(sbs, [128, 1], F32, "m1")
                nc.vector.reduce_max(m1[:], p_sb[:], axis=mybir.AxisListType.X)
                mk1 = _mktile(sbs, [128, NE], F32, "mk1")
                nc.vector.tensor_scalar(mk1[:], p_sb[:], m1[:, :1], None,
                                        op0=ALU.is_equal)
                p2 = _mktile(sbs, [128, NE], F32, "p2")
                nc.vector.scalar_tensor_tensor(p2[:], mk1[:], -1e9, p_sb[:],
                                               op0=ALU.mult, op1=ALU.add)
                m2 = _mktile(sbs, [128, 1], F32, "m2")
                nc.vector.reduce_max(m2[:], p2[:], axis=mybir.AxisListType.X)
                mk2 = _mktile(sbs, [128, NE], F32, "mk2")
                nc.vector.tensor_scalar(mk2[:], p2[:], m2[:, :1], None,
                                        op0=ALU.is_equal)
                msum = _mktile(sbs, [128, 1], F32, "msum")
                nc.vector.tensor_tensor(msum[:], m1[:], m2[:], op=ALU.add)
                rw = _mktile(sbs, [128, 1], F32, "rw")
                nc.vector.reciprocal(rw[:], msum[:])
                mks = _mktile(sbs, [128, NE], F32, "mks")
                nc.vector.tensor_tensor(mks[:], mk1[:], mk2[:], op=ALU.add)
                pw = _mktile(sbs, [128, NE], F32, "pw")
                nc.vector.tensor_tensor(pw[:], p_sb[:], mks[:], op=ALU.mult)
                nc.vector.tensor_scalar_mul(W_sb[:, n, :], pw[:], rw[:, :1])

            # ---- MoE (dense-masked v1) ----
            acc = _mktile(sb, [128, NT, C], F32, "acc")
            for e in range(NE):
                u = [_mktile(sbu, [128, T], BF16, "u") for _ in range(MT)]
                for mt in range(MT):
                    p1 = _mktile(psA, [128, T], F32, "A")
                    p3 = _mktile(psA, [128, T], F32, "A")
                    for kt in range(KT):
                        nc.tensor.matmul(p1[:], w1_sb[:, e * KT + kt,
                                                      bass.ts(mt, 128)],
                                         h2T[kt][:], start=(kt == 0),
                                         stop=(kt == KT - 1))
                    for kt in range(KT):
                        nc.tensor.matmul(p3[:], w3_sb[:, e * KT + kt,
                                                      bass.ts(mt, 128)],
                                         h2T[kt][:], start=(kt == 0),
                                         stop=(kt == KT - 1))
                    g_sb = _mktile(sbs, [128, T], BF16, "g")
                    nc.scalar.activation(g_sb[:], p1[:], AF.Silu)
                    nc.vector.tensor_tensor(u[mt][:], g_sb[:], p3[:], op=ALU.mult)
                for n in range(NT):
                    px = _mktile(psO, [128, C], F32, "O")
                    for mt in range(MT):
                        nc.tensor.matmul(px[:], u[mt][:, bass.ts(n, 128)],
                                         w2_sb[:, e * MT + mt, :],
                                         start=(mt == 0), stop=(mt == MT - 1))
                    base = x2_sb if e == 0 else acc
                    nc.vector.scalar_tensor_tensor(
                        acc[:, n, :], px[:], W_sb[:, n, e:e + 1], base[:, n, :],
                        op0=ALU.mult, op1=ALU.add)

            # ---- store ----
            nc.sync.dma_start(out_d[b].rearrange("(n p) c -> p n c", p=128),
                              acc[:])

    nc.finalize()
    return nc


def _host_prep(inputs):
    """Fold ln weights into the adjacent matmul weights, transpose to
    [contract, out] layouts, pad the MoE hidden dim 682 -> 768, and build
    rope/mask/permutation constant tables."""
    f32 = np.float32
    bf = ml_dtypes.bfloat16
    x = np.asarray(inputs["x"], f32)
    ln1 = np.asarray(inputs["ln1_w"], f32)
    ln2 = np.asarray(inputs["ln2_w"], f32)
    wq = np.asarray(inputs["wq"], f32) * ln1[None, :]
    wk = np.asarray(inputs["wk"], f32) * ln1[None, :]
    wv = np.asarray(inputs["wv"], f32) * ln1[None, :]
    wo = np.asarray(inputs["wo"], f32)
    rw = np.asarray(inputs["router_w"], f32) * ln2[None, :]
    w1 = np.asarray(inputs["w1"], f32) * ln2[None, None, :]
    w3 = np.asarray(inputs["w3"], f32) * ln2[None, None, :]
    w2 = np.asarray(inputs["w2"], f32)

    rt = rw.T                                   # [C, NE] f32
    rt_hi = rt.astype(bf)
    rt_lo = (rt - rt_hi.astype(f32)).astype(bf)

    w1t = np.zeros((NE, C, HIDP), bf)
    w3t = np.zeros((NE, C, HIDP), bf)
    w2t = np.zeros((NE, HIDP, C), bf)
    w1t[:, :, :HID] = w1.transpose(0, 2, 1).astype(bf)
    w3t[:, :, :HID] = w3.transpose(0, 2, 1).astype(bf)
    w2t[:, :HID, :] = w2.transpose(0, 2, 1).astype(bf)

    inv_freq = 1.0 / (10000.0 ** (np.arange(0, HD, 2, dtype=f32) / HD))
    t = np.arange(T, dtype=f32)
    freqs = np.outer(t, inv_freq)
    emb = np.concatenate([freqs, freqs], axis=-1)       # [T, HD]
    cosT = np.tile(np.cos(emb).T, (2, 1)).astype(f32)   # [128, T]
    sinT = np.tile(np.sin(emb).T, (2, 1)).astype(f32)
    scale = 1.0 / np.sqrt(np.float32(HD))

    prot = np.zeros((128, 128), f32)
    hh = HD // 2
    for blk in (0, 64):
        for m in range(hh):
            prot[blk + m + hh, blk + m] = -1.0
        for m in range(hh, HD):
            prot[blk + m - hh, blk + m] = 1.0

    mdiag = np.where(np.arange(128)[None, :] <= np.arange(128)[:, None],
                     np.float32(0.0), np.float32(NEG)).astype(f32)

    common = {
        "wqt": wq.T.astype(bf), "wkt": wk.T.astype(bf),
        "wvt": wv.T.astype(bf), "wot": wo.T.astype(bf),
        "rt_hi": rt_hi, "rt_lo": rt_lo,
        "w1t": w1t, "w3t": w3t, "w2t": w2t,
        "cosq": (cosT * scale).astype(f32), "sinq": (sinT * scale).astype(f32),
        "cosk": cosT, "sink": sinT,
        "prot": prot.astype(bf), "ident": np.eye(128, dtype=f32).astype(bf),
        "mdiag": mdiag,
    }
    return x, common


_NC_CACHE = {}


def _get_nc(n_batches=B_LOC):
    if n_batches not in _NC_CACHE:
        _NC_CACHE[n_batches] = _build(n_batches)
    return _NC_CACHE[n_batches]


def run(inputs, trace=False):
    x, common = _host_prep(inputs)
    in_maps = [dict(common, x=x[c * B_LOC:(c + 1) * B_LOC])
               for c in range(N_CORES)]
    nc = _get_nc()
    res = run_bass_kernel_spmd(nc, in_maps, list(range(N_CORES)), trace=trace)
    out = np.concatenate([res.results[c]["out"] for c in range(N_CORES)], axis=0)
    return np.ascontiguousarray(out.astype(np.float32)), res


def kernel(**inputs):
    out, _ = run(inputs)
    return out

